# revision 15
# baseline (speedup 1.0000x reference)
"""Trainium2 Bass kernel for nn_Aggregator (GNN message passing).

Computation per (b, e):
  scores[k] = <side[b], rel[b,e,k,:]>          (contract over D=64)
  attn      = softmax_k(scores)
  agg[d]    = sum_k attn[k] * nbr[b,e,k,d]     (contract over K=32)
  out       = relu(cat(self[b,e], agg) @ W + bias)

Sharding: data-parallel over the leading batch dim B=1024 across 8 cores
(128 batches/core); weights replicated.

Per-core design (fp16 on the wire, ~34.2 MB DMA/core, DMA-bound):
  - partitions = the 128 batches of the core
  - scores on PE: 64 accumulating diag(side[:,d]) matmuls per phase
  - softmax: exp on ACT (bf16 for range), row-sum + recip + esc*1/sum on DVE
  - p2 = esc_n * nbr on DVE (fp16 2x mode, k-innermost layout)
  - k-sum on PE: 32 accumulating identity matmuls over stride-K slices of p2
    into PSUM fp32 (frees the DVE, which was the baseline bottleneck)
  - linear on PE: per-e transposes of agg -> xt (ACT copies), bias as
    ones x bvec matmul, relu + downcast on ACT, per-unit output DMA
  - phases [2,8,8,8,4,2] e's; rel streams one phase ahead of nbr so scores
    for phase i+1 overlap aggs of phase i; diag DMA'd in two halves so the
    first scores start early; dummy exp preloads the ACT table
"""

import numpy as np

B, E, K, D = 1024, 32, 32, 64
NCORES = 8
BC = B // NCORES   # 128 batches per core

# phases (e-counts) and agg units (4e max); small edges shrink ramp + tail
PH = [2, 8, 8, 8, 4, 2]
NPH = len(PH)
PE0 = [sum(PH[:i]) for i in range(NPH)]   # phase start e
RELCH = [2 if p == 8 else 1 for p in PH]  # d-chunks per phase (32d or 64d)


def _units(ph):
    """Agg units (local_e_offset, ue) within phase ph."""
    pe = PH[ph]
    if pe <= 4:
        return [(0, pe)]
    return [(i, 4) for i in range(0, pe, 4)]


# which units run their k-sum on DVE (tree) instead of PE; strided-rhs
# identity matmuls measured ~5.6x slower than contiguous, so all units
# tree on DVE (DVE ~75us still sits under the ~86us DMA stream)
DVE_TREE_UNITS = {(ph, eL) for ph in range(NPH) for eL, _ in _units(ph)}

_CACHE = {}


def _build_nc():
    from contextlib import ExitStack

    import concourse.bass as bass
    import concourse.bacc as bacc
    import concourse.tile as tile
    from concourse import mybir

    f32 = mybir.dt.float32
    f16 = mybir.dt.float16
    bf16 = mybir.dt.bfloat16
    Alu = mybir.AluOpType
    Act = mybir.ActivationFunctionType

    nc = bacc.Bacc()

    # HBM layouts (host-packed, fp16):
    #   rel chunks: [b, (ds, e, k)] per (phase, chunk), d split per RELCH
    #   nbr units:  [b, (eu, d, k)] = nbr[b, e0+eu, k, d]   (k innermost)
    #   diag lo/hi: [b, (d, f)] = side[b, d] * delta(b, f), d<32 / d>=32
    #   selft[d, (e, b)] = self[b, e, d]
    rel_h = {}
    ci = 0
    for ph in range(NPH):
        for c in range(RELCH[ph]):
            sz = (64 // RELCH[ph]) * PH[ph] * K
            rel_h[ph, c] = nc.declare_dram_parameter(f"rel{ci}", [BC, sz], f16,
                                                     isOutput=False)
            ci += 1
    nbr_h = {}
    ui = 0
    for ph in range(NPH):
        for eL, ue in _units(ph):
            nbr_h[ph, eL] = nc.declare_dram_parameter(
                f"nbr{ui}", [BC, ue * D * K], f16, isOutput=False)
            ui += 1
    dglo_h = nc.declare_dram_parameter("dglo", [BC, 32 * 128], f16, isOutput=False)
    dghi_h = nc.declare_dram_parameter("dghi", [BC, 32 * 128], f16, isOutput=False)
    selft_h = nc.declare_dram_parameter("selft", [D, E * BC], f16, isOutput=False)
    wf_h = nc.declare_dram_parameter("wful", [2 * D, D], f16, isOutput=False)
    b_h = nc.declare_dram_parameter("bvec", [1, 512], f16, isOutput=False)
    ones_h = nc.declare_dram_parameter("ones", [1, 128], f16, isOutput=False)
    iden_h = nc.declare_dram_parameter("iden", [128, 128], f16, isOutput=False)
    out_h = nc.declare_dram_parameter("outh", [BC, E * D], f16, isOutput=True)

    def vw(t, off_el, dims):
        """View of tile/AP t at extra element offset with given free dims."""
        a = t[:] if hasattr(t, "tile_id") else t
        return bass.AP(tensor=a.tensor, offset=a.offset + off_el, ap=[a.ap[0]] + dims)

    with tile.TileContext(nc) as tc, ExitStack() as ctx:
        consts = ctx.enter_context(tc.tile_pool(name="consts", bufs=1))
        rel8_pool = ctx.enter_context(tc.tile_pool(name="rel8", bufs=2))
        rel2_pool = ctx.enter_context(tc.tile_pool(name="rel2", bufs=2))
        nbr4_pool = ctx.enter_context(tc.tile_pool(name="nbr4", bufs=2))
        nbr2_pool = ctx.enter_context(tc.tile_pool(name="nbr2", bufs=2))
        p2_pool = ctx.enter_context(tc.tile_pool(name="p2", bufs=1))
        tmp_pool = ctx.enter_context(tc.tile_pool(name="tmp", bufs=2))
        work = ctx.enter_context(tc.tile_pool(name="work", bufs=1))
        agg_pool = ctx.enter_context(tc.tile_pool(name="agg", bufs=3))
        ps_sc = ctx.enter_context(tc.tile_pool(name="ps_sc", bufs=2, space="PSUM"))
        ps_ag = ctx.enter_context(tc.tile_pool(name="ps_ag", bufs=2, space="PSUM"))
        ps_tr = ctx.enter_context(tc.tile_pool(name="ps_tr", bufs=2, space="PSUM"))
        ps_lin = ctx.enter_context(tc.tile_pool(name="ps_lin", bufs=2, space="PSUM"))
        ps_warm = ctx.enter_context(tc.tile_pool(name="ps_warm", bufs=2, space="PSUM"))

        # ---- constants + streamed inputs on one sync queue, arrival order ----
        ones_sb = consts.tile([1, 128], f16)
        nc.sync.dma_start(out=ones_sb, in_=ones_h[:])
        wf_sb = consts.tile([2 * D, D], f16)
        nc.sync.dma_start(out=wf_sb, in_=wf_h[:])
        bvec_sb = consts.tile([1, 512], f16)
        nc.sync.dma_start(out=bvec_sb, in_=b_h[:])
        iden_sb = consts.tile([128, 128], f16)
        nc.sync.dma_start(out=iden_sb, in_=iden_h[:])
        diag_sb = consts.tile([BC, D * 128], f16)
        nc.sync.dma_start(out=diag_sb[:, 0 : 32 * 128], in_=dglo_h[:])

        rel_t = {}
        nbr_t = {}
        warm_jobs = []

        def warm_mm(src):
            """Tiny PE matmul reading src: keeps the HAM activity window from
            going fully idle so the PE stays at K=8/8 between score bursts."""
            wp = ps_warm.tile([1, 8], f32, tag="wm")
            nc.tensor.matmul(
                out=vw(wp, 0, [[1, 1]]),
                lhsT=iden_sb[:, 0:1],
                rhs=vw(src, 0, [[1, 1]]),
                start=True,
                stop=True,
            )

        def dma_rel(ph):
            for c in range(RELCH[ph]):
                sz = (64 // RELCH[ph]) * PH[ph] * K
                pool = rel8_pool if sz == 8192 else rel2_pool
                t = pool.tile([BC, sz], f16, tag=f"rel{sz}")
                nc.sync.dma_start(out=t, in_=rel_h[ph, c][:])
                warm_jobs.append(t)
                rel_t[ph, c] = t

        def dma_nbr(ph):
            for eL, ue in _units(ph):
                pool = nbr4_pool if ue == 4 else nbr2_pool
                t = pool.tile([BC, ue * D * K], f16, tag=f"nbr{ue}")
                nc.sync.dma_start(out=t, in_=nbr_h[ph, eL][:])
                warm_jobs.append(t)
                nbr_t[ph, eL] = t

        # rel one phase ahead of nbr; diag-hi after first rel; selft mid-ramp
        dma_rel(0)
        nc.sync.dma_start(out=diag_sb[:, 32 * 128 :], in_=dghi_h[:])
        dma_nbr(0)
        dma_rel(1)
        # xt[0:64, e*128:+128] = selfT_e ; rows 64:128 get aggT_e per unit
        xt = consts.tile([2 * D, E * BC], f16)
        nc.sync.dma_start(out=xt[0:D], in_=selft_h[:])
        for ph in range(1, NPH):
            if ph + 1 < NPH:
                dma_rel(ph + 1)
            dma_nbr(ph)

        out_all = work.tile([BC, E * D], f16)

        # ---- ACT exp-table preload (overlaps the DMA head) ----
        warm = work.tile([1, 128], bf16, tag="warm")
        nc.scalar.activation(out=warm, in_=ones_sb, func=Act.Exp)

        # ---- pipeline: per phase scores/softmax, per unit agg/linear ----
        for ph in range(NPH):
            pe = PH[ph]
            nsc = pe * K
            dpc = 64 // RELCH[ph]
            warm_mm(rel_t[ph, 0])
            sc_ps = ps_sc.tile([BC, nsc], f32, tag="sc")
            for c in range(RELCH[ph]):
                rt = rel_t[ph, c]
                for ds in range(dpc):
                    d = dpc * c + ds
                    nc.tensor.matmul(
                        out=sc_ps,
                        lhsT=vw(diag_sb, d * 128, [[1, 128]]),
                        rhs=vw(rt, ds * nsc, [[1, nsc]]),
                        start=(d == 0),
                        stop=(d == 63),
                    )
            esc = work.tile([BC, nsc], bf16, tag=f"esc{ph}")
            nc.scalar.activation(out=esc, in_=sc_ps, func=Act.Exp)
            warm_mm(esc)
            sums = work.tile([BC, pe], f32, tag=f"sums{ph}")
            nc.vector.tensor_reduce(
                out=sums,
                in_=vw(esc, 0, [[K, pe], [1, K]]),
                axis=mybir.AxisListType.X,
                op=Alu.add,
            )
            rs = work.tile([BC, pe], f32, tag=f"rs{ph}")
            nc.vector.reciprocal(out=rs, in_=sums)
            esc_n = work.tile([BC, nsc], f16, tag=f"escn{ph}")
            nc.vector.tensor_mul(
                out=vw(esc_n, 0, [[K, pe], [1, K]]),
                in0=vw(esc, 0, [[K, pe], [1, K]]),
                in1=vw(rs, 0, [[1, pe], [0, K]]),
            )

            for eL, ue in _units(ph):
                e0u = PE0[ph] + eL
                uid = (ph, eL)
                p2 = p2_pool.tile([BC, ue * D * K], f16, tag=f"p2{ue}")
                nc.vector.tensor_mul(
                    out=vw(p2, 0, [[D * K, ue], [K, D], [1, K]]),
                    in0=vw(nbr_t[ph, eL], 0, [[D * K, ue], [K, D], [1, K]]),
                    in1=vw(esc_n, eL * K, [[K, ue], [0, D], [1, K]]),
                )
                agg = agg_pool.tile([BC, ue * D], f16, tag="agg")
                if uid in DVE_TREE_UNITS:
                    warm_mm(p2)
                    src, kk = p2, K
                    while kk > 2:
                        kk //= 2
                        dst = tmp_pool.tile([BC, ue * D * kk], f16, tag="tr")
                        nc.vector.tensor_add(
                            out=vw(dst, 0, [[kk, ue * D], [1, kk]]),
                            in0=vw(src, 0, [[2 * kk, ue * D], [1, kk]]),
                            in1=vw(src, kk, [[2 * kk, ue * D], [1, kk]]),
                        )
                        if kk == 16:
                            warm_mm(dst)
                        src = dst
                    nc.vector.tensor_add(
                        out=vw(agg, 0, [[1, ue * D]]),
                        in0=vw(src, 0, [[2, ue * D]]),
                        in1=vw(src, 1, [[2, ue * D]]),
                    )
                else:
                    ag_ps = ps_ag.tile([BC, ue * D], f32, tag="ag")
                    for k in range(K):
                        nc.tensor.matmul(
                            out=ag_ps,
                            lhsT=iden_sb,
                            rhs=vw(p2, k, [[K, ue * D]]),
                            start=(k == 0),
                            stop=(k == K - 1),
                        )
                    nc.scalar.copy(out=agg, in_=ag_ps)

                for c in range(ue // 2):
                    tp = ps_tr.tile([128, 128], f16, tag="tp")
                    nc.tensor.transpose(
                        out=tp, in_=vw(agg, c * 128, [[1, 128]]), identity=iden_sb
                    )
                    e0 = e0u + 2 * c
                    nc.scalar.copy(
                        out=xt[D : 2 * D, e0 * BC : (e0 + 1) * BC], in_=tp[0:D]
                    )
                    nc.scalar.copy(
                        out=xt[D : 2 * D, (e0 + 1) * BC : (e0 + 2) * BC],
                        in_=tp[D : 2 * D],
                    )
                lin = ps_lin.tile([BC, ue * D], f32, tag="lin")
                nc.tensor.matmul(
                    out=lin, lhsT=ones_sb, rhs=bvec_sb[:, 0 : ue * D],
                    start=True, stop=True,
                )
                for i in range(ue):
                    e = e0u + i
                    nc.tensor.matmul(
                        out=vw(lin, i * D, [[1, D]]),
                        lhsT=xt[:, e * BC : (e + 1) * BC],
                        rhs=wf_sb,
                        start=False,
                        stop=True,
                        skip_group_check=True,
                    )
                nc.scalar.activation(
                    out=vw(out_all, e0u * D, [[1, ue * D]]),
                    in_=lin,
                    func=Act.Relu,
                )
                # ACT's separate HWDGE ring: not FIFO-blocked behind the
                # input stream on the sync ring
                nc.scalar.dma_start(
                    out=vw(out_h[:], e0u * D, [[1, ue * D]]),
                    in_=vw(out_all, e0u * D, [[1, ue * D]]),
                )

    nc.finalize()
    return nc


def _get_nc():
    if "nc" not in _CACHE:
        _CACHE["nc"] = _build_nc()
    return _CACHE["nc"]


def _make_in_maps(self_vectors, neighbor_vectors, neighbor_relations,
                  side_embeddings, W, b):
    f16 = np.float16
    iden = np.eye(128, dtype=f16)
    ones = np.ones((1, 128), dtype=f16)
    wful = np.ascontiguousarray(np.asarray(W, dtype=f16))
    bvec = np.ascontiguousarray(np.tile(np.asarray(b, dtype=f16), 8)).reshape(1, 512)
    rel = np.asarray(neighbor_relations, dtype=f16)
    nbr = np.asarray(neighbor_vectors, dtype=f16)
    sv = np.asarray(self_vectors, dtype=f16)
    side = np.asarray(side_embeddings, dtype=np.float32)

    in_maps = []
    for cc in range(NCORES):
        sl = slice(cc * BC, (cc + 1) * BC)
        m = {}
        rc = rel[sl]                                   # [BC, E, K, D]
        ci = 0
        for ph in range(NPH):
            pe, e0 = PH[ph], PE0[ph]
            dpc = 64 // RELCH[ph]
            blk = rc[:, e0 : e0 + pe]                  # b e k d
            for c in range(RELCH[ph]):
                sub = blk[:, :, :, c * dpc : (c + 1) * dpc]  # b e k ds
                m[f"rel{ci}"] = np.ascontiguousarray(
                    sub.transpose(0, 3, 1, 2)).reshape(BC, dpc * pe * K)
                ci += 1
        nc_ = nbr[sl]                                  # [BC, E, K, D]
        ui = 0
        for ph in range(NPH):
            for eL, ue in _units(ph):
                e0 = PE0[ph] + eL
                sub = nc_[:, e0 : e0 + ue]             # b eu k d
                m[f"nbr{ui}"] = np.ascontiguousarray(
                    sub.transpose(0, 1, 3, 2)).reshape(BC, ue * D * K)
                ui += 1
        dg = np.zeros((BC, D, 128), dtype=f16)
        dg[np.arange(BC), :, np.arange(BC)] = side[sl].astype(f16)
        m["dglo"] = np.ascontiguousarray(dg[:, 0:32]).reshape(BC, 32 * 128)
        m["dghi"] = np.ascontiguousarray(dg[:, 32:64]).reshape(BC, 32 * 128)
        m["selft"] = np.ascontiguousarray(sv[sl].transpose(2, 1, 0)).reshape(D, E * BC)
        m["wful"] = wful
        m["bvec"] = bvec
        m["ones"] = ones
        m["iden"] = iden
        in_maps.append(m)
    return in_maps


def kernel(self_vectors, neighbor_vectors, neighbor_relations, side_embeddings, W, b,
           _trace=False, _tmpdir=None):
    from concourse import bass_utils

    nc = _get_nc()
    in_maps = _make_in_maps(
        self_vectors, neighbor_vectors, neighbor_relations, side_embeddings, W, b
    )
    res = bass_utils.run_bass_kernel_spmd(
        nc, in_maps, list(range(NCORES)), trace=_trace, tmpdir=_tmpdir
    )
    _CACHE["last_results"] = res
    out = np.concatenate(
        [
            res.results[c]["outh"].astype(np.float32).reshape(BC, E, D)
            for c in range(NCORES)
        ],
        axis=0,
    )
    return out


# revision 20
# speedup vs baseline: 1.0061x; 1.0061x over previous
"""Trainium2 Bass kernel for nn_Aggregator (GNN message passing).

Computation per (b, e):
  scores[k] = <side[b], rel[b,e,k,:]>          (contract over D=64)
  attn      = softmax_k(scores)
  agg[d]    = sum_k attn[k] * nbr[b,e,k,d]     (contract over K=32)
  out       = relu(cat(self[b,e], agg) @ W + bias)

Sharding: data-parallel over the leading batch dim B=1024 across 8 cores
(128 batches/core); weights replicated.

Per-core design (fp16 on the wire, ~34.2 MB DMA/core, DMA-bound):
  - partitions = the 128 batches of the core
  - scores on PE: 64 accumulating diag(side[:,d]) matmuls per phase
  - softmax: exp on ACT (bf16 for range), row-sum + recip + esc*1/sum on DVE
  - p2 = esc_n * nbr on DVE (fp16 2x mode, k-innermost layout)
  - k-sum on PE: 32 accumulating identity matmuls over stride-K slices of p2
    into PSUM fp32 (frees the DVE, which was the baseline bottleneck)
  - linear on PE: per-e transposes of agg -> xt (ACT copies), bias as
    ones x bvec matmul, relu + downcast on ACT, per-unit output DMA
  - phases [2,8,8,8,4,2] e's; rel streams one phase ahead of nbr so scores
    for phase i+1 overlap aggs of phase i; diag DMA'd in two halves so the
    first scores start early; dummy exp preloads the ACT table
"""

import numpy as np

B, E, K, D = 1024, 32, 32, 64
NCORES = 8
BC = B // NCORES   # 128 batches per core

# phases (e-counts) and agg units (4e max); small edges shrink ramp + tail
PH = [2, 8, 8, 8, 4, 2]
NPH = len(PH)
PE0 = [sum(PH[:i]) for i in range(NPH)]   # phase start e
RELCH = [2 if p == 8 else 1 for p in PH]  # d-chunks per phase (32d or 64d)


def _units(ph):
    """Agg units (local_e_offset, ue) within phase ph."""
    pe = PH[ph]
    if pe <= 4:
        return [(0, pe)]
    return [(i, 4) for i in range(0, pe, 4)]


# which units run their k-sum on DVE (tree) instead of PE; strided-rhs
# identity matmuls measured ~5.6x slower than contiguous, so all units
# tree on DVE (DVE ~75us still sits under the ~86us DMA stream)
DVE_TREE_UNITS = {(ph, eL) for ph in range(NPH) for eL, _ in _units(ph)}

_CACHE = {}


def _build_nc():
    from contextlib import ExitStack

    import concourse.bass as bass
    import concourse.bacc as bacc
    import concourse.tile as tile
    from concourse import mybir

    f32 = mybir.dt.float32
    f16 = mybir.dt.float16
    bf16 = mybir.dt.bfloat16
    Alu = mybir.AluOpType
    Act = mybir.ActivationFunctionType

    nc = bacc.Bacc()

    # HBM layouts (host-packed, fp16):
    #   rel chunks: [b, (ds, e, k)] per (phase, chunk), d split per RELCH
    #   nbr units:  [b, (eu, d, k)] = nbr[b, e0+eu, k, d]   (k innermost)
    #   diag lo/hi: [b, (d, f)] = side[b, d] * delta(b, f), d<32 / d>=32
    #   selft[d, (e, b)] = self[b, e, d]
    rel_h = {}
    ci = 0
    for ph in range(NPH):
        for c in range(RELCH[ph]):
            sz = (64 // RELCH[ph]) * PH[ph] * K
            rel_h[ph, c] = nc.declare_dram_parameter(f"rel{ci}", [BC, sz], f16,
                                                     isOutput=False)
            ci += 1
    nbr_h = {}
    ui = 0
    for ph in range(NPH):
        for eL, ue in _units(ph):
            nbr_h[ph, eL] = nc.declare_dram_parameter(
                f"nbr{ui}", [BC, ue * D * K], f16, isOutput=False)
            ui += 1
    # quarter-tiled diag: dq[p, d*32 + f] = side[p, d] * delta(p % 32, f)
    dglo_h = nc.declare_dram_parameter("dglo", [BC, 32 * 32], f16, isOutput=False)
    dghi_h = nc.declare_dram_parameter("dghi", [BC, 32 * 32], f16, isOutput=False)
    selft_h = nc.declare_dram_parameter("selft", [D, E * BC], f16, isOutput=False)
    wf_h = nc.declare_dram_parameter("wful", [2 * D, D], f16, isOutput=False)
    b_h = nc.declare_dram_parameter("bvec", [1, 512], f16, isOutput=False)
    ones_h = nc.declare_dram_parameter("ones", [1, 128], f16, isOutput=False)
    iden_h = nc.declare_dram_parameter("iden", [128, 128], f16, isOutput=False)
    out_h = nc.declare_dram_parameter("outh", [BC, E * D], f16, isOutput=True)

    def vw(t, off_el, dims):
        """View of tile/AP t at extra element offset with given free dims."""
        a = t[:] if hasattr(t, "tile_id") else t
        return bass.AP(tensor=a.tensor, offset=a.offset + off_el, ap=[a.ap[0]] + dims)

    with tile.TileContext(nc) as tc, ExitStack() as ctx:
        consts = ctx.enter_context(tc.tile_pool(name="consts", bufs=1))
        rel8_pool = ctx.enter_context(tc.tile_pool(name="rel8", bufs=2))
        rel2_pool = ctx.enter_context(tc.tile_pool(name="rel2", bufs=2))
        nbr4_pool = ctx.enter_context(tc.tile_pool(name="nbr4", bufs=2))
        nbr2_pool = ctx.enter_context(tc.tile_pool(name="nbr2", bufs=2))
        p2_pool = ctx.enter_context(tc.tile_pool(name="p2", bufs=1))
        tmp_pool = ctx.enter_context(tc.tile_pool(name="tmp", bufs=2))
        work = ctx.enter_context(tc.tile_pool(name="work", bufs=1))
        agg_pool = ctx.enter_context(tc.tile_pool(name="agg", bufs=3))
        ps_sc = ctx.enter_context(tc.tile_pool(name="ps_sc", bufs=2, space="PSUM"))
        ps_ag = ctx.enter_context(tc.tile_pool(name="ps_ag", bufs=2, space="PSUM"))
        ps_tr = ctx.enter_context(tc.tile_pool(name="ps_tr", bufs=2, space="PSUM"))
        ps_lin = ctx.enter_context(tc.tile_pool(name="ps_lin", bufs=2, space="PSUM"))
        ps_warm = ctx.enter_context(tc.tile_pool(name="ps_warm", bufs=2, space="PSUM"))

        # ---- constants + streamed inputs on one sync queue, arrival order ----
        ones_sb = consts.tile([1, 128], f16)
        nc.sync.dma_start(out=ones_sb, in_=ones_h[:])
        wf_sb = consts.tile([2 * D, D], f16)
        nc.sync.dma_start(out=wf_sb, in_=wf_h[:])
        bvec_sb = consts.tile([1, 512], f16)
        nc.sync.dma_start(out=bvec_sb, in_=b_h[:])
        iden_sb = consts.tile([128, 128], f16)
        nc.sync.dma_start(out=iden_sb, in_=iden_h[:])
        diag_sb = consts.tile([BC, D * 32], f16)
        nc.sync.dma_start(out=diag_sb[:, 0 : 32 * 32], in_=dglo_h[:])

        rel_t = {}
        nbr_t = {}
        warm_jobs = []

        def warm_mm(src):
            """Tiny PE matmul reading src: keeps the HAM activity window from
            going fully idle so the PE stays at K=8/8 between score bursts."""
            wp = ps_warm.tile([1, 8], f32, tag="wm")
            nc.tensor.matmul(
                out=vw(wp, 0, [[1, 1]]),
                lhsT=iden_sb[:, 0:1],
                rhs=vw(src, 0, [[1, 1]]),
                start=True,
                stop=True,
            )

        def dma_rel(ph):
            for c in range(RELCH[ph]):
                sz = (64 // RELCH[ph]) * PH[ph] * K
                pool = rel8_pool if sz == 8192 else rel2_pool
                t = pool.tile([BC, sz], f16, tag=f"rel{sz}")
                nc.sync.dma_start(out=t, in_=rel_h[ph, c][:])
                warm_jobs.append(t)
                rel_t[ph, c] = t

        def dma_nbr(ph):
            for eL, ue in _units(ph):
                pool = nbr4_pool if ue == 4 else nbr2_pool
                t = pool.tile([BC, ue * D * K], f16, tag=f"nbr{ue}")
                nc.sync.dma_start(out=t, in_=nbr_h[ph, eL][:])
                warm_jobs.append(t)
                nbr_t[ph, eL] = t

        # rel one phase ahead of nbr; diag-hi after first rel; selft mid-ramp
        dma_rel(0)
        nc.sync.dma_start(out=diag_sb[:, 32 * 32 :], in_=dghi_h[:])
        dma_nbr(0)
        dma_rel(1)
        # xt[0:64, e*128:+128] = selfT_e ; rows 64:128 get aggT_e per unit
        xt = consts.tile([2 * D, E * BC], f16)
        nc.sync.dma_start(out=xt[0:D], in_=selft_h[:])
        for ph in range(1, NPH):
            if ph + 1 < NPH:
                dma_rel(ph + 1)
            dma_nbr(ph)

        out_all = work.tile([BC, E * D], f16)

        # ---- ACT exp-table preload (overlaps the DMA head) ----
        warm = work.tile([1, 128], bf16, tag="warm")
        nc.scalar.activation(out=warm, in_=ones_sb, func=Act.Exp)

        # ---- pipeline: per phase scores/softmax, per unit agg/linear ----
        for ph in range(NPH):
            pe = PH[ph]
            nsc = pe * K
            dpc = 64 // RELCH[ph]
            warm_mm(rel_t[ph, 0])
            sc_ps = ps_sc.tile([BC, nsc], f32, tag="sc")
            # 4 concurrent 32x32-row-group chains: short LDWEIGHTS (32 cols)
            # that overlap other quarters' matmuls, and a 4x shorter critical
            # accumulation chain per quarter
            for c in range(RELCH[ph]):
                rt = rel_t[ph, c]
                for ds in range(dpc):
                    d = dpc * c + ds
                    for q in range(4):
                        nc.tensor.matmul(
                            out=sc_ps[32 * q : 32 * (q + 1), :],
                            lhsT=diag_sb[32 * q : 32 * (q + 1), d * 32 : (d + 1) * 32],
                            rhs=rt[32 * q : 32 * (q + 1), ds * nsc : (ds + 1) * nsc],
                            start=(d == 0),
                            stop=(d == 63),
                            tile_position=(32 * q, 32 * q),
                            skip_group_check=True,
                        )
            esc = work.tile([BC, nsc], bf16, tag=f"esc{ph}")
            nc.scalar.activation(out=esc, in_=sc_ps, func=Act.Exp)
            warm_mm(esc)
            sums = work.tile([BC, pe], f32, tag=f"sums{ph}")
            nc.vector.tensor_reduce(
                out=sums,
                in_=vw(esc, 0, [[K, pe], [1, K]]),
                axis=mybir.AxisListType.X,
                op=Alu.add,
            )
            rs = work.tile([BC, pe], f32, tag=f"rs{ph}")
            nc.vector.reciprocal(out=rs, in_=sums)
            esc_n = work.tile([BC, nsc], f16, tag=f"escn{ph}")
            nc.vector.tensor_mul(
                out=vw(esc_n, 0, [[K, pe], [1, K]]),
                in0=vw(esc, 0, [[K, pe], [1, K]]),
                in1=vw(rs, 0, [[1, pe], [0, K]]),
            )

            for eL, ue in _units(ph):
                e0u = PE0[ph] + eL
                uid = (ph, eL)
                p2 = p2_pool.tile([BC, ue * D * K], f16, tag=f"p2{ue}")
                nc.vector.tensor_mul(
                    out=vw(p2, 0, [[D * K, ue], [K, D], [1, K]]),
                    in0=vw(nbr_t[ph, eL], 0, [[D * K, ue], [K, D], [1, K]]),
                    in1=vw(esc_n, eL * K, [[K, ue], [0, D], [1, K]]),
                )
                agg = agg_pool.tile([BC, ue * D], f16, tag="agg")
                if uid in DVE_TREE_UNITS:
                    warm_mm(p2)
                    src, kk = p2, K
                    while kk > 2:
                        kk //= 2
                        dst = tmp_pool.tile([BC, ue * D * kk], f16, tag="tr")
                        nc.vector.tensor_add(
                            out=vw(dst, 0, [[kk, ue * D], [1, kk]]),
                            in0=vw(src, 0, [[2 * kk, ue * D], [1, kk]]),
                            in1=vw(src, kk, [[2 * kk, ue * D], [1, kk]]),
                        )
                        if kk == 16:
                            warm_mm(dst)
                        src = dst
                    nc.vector.tensor_add(
                        out=vw(agg, 0, [[1, ue * D]]),
                        in0=vw(src, 0, [[2, ue * D]]),
                        in1=vw(src, 1, [[2, ue * D]]),
                    )
                else:
                    ag_ps = ps_ag.tile([BC, ue * D], f32, tag="ag")
                    for k in range(K):
                        nc.tensor.matmul(
                            out=ag_ps,
                            lhsT=iden_sb,
                            rhs=vw(p2, k, [[K, ue * D]]),
                            start=(k == 0),
                            stop=(k == K - 1),
                        )
                    nc.scalar.copy(out=agg, in_=ag_ps)

                for c in range(ue // 2):
                    tp = ps_tr.tile([128, 128], f16, tag="tp")
                    nc.tensor.transpose(
                        out=tp, in_=vw(agg, c * 128, [[1, 128]]), identity=iden_sb
                    )
                    e0 = e0u + 2 * c
                    nc.scalar.copy(
                        out=xt[D : 2 * D, e0 * BC : (e0 + 1) * BC], in_=tp[0:D]
                    )
                    nc.scalar.copy(
                        out=xt[D : 2 * D, (e0 + 1) * BC : (e0 + 2) * BC],
                        in_=tp[D : 2 * D],
                    )
                lin = ps_lin.tile([BC, ue * D], f32, tag="lin")
                nc.tensor.matmul(
                    out=lin, lhsT=ones_sb, rhs=bvec_sb[:, 0 : ue * D],
                    start=True, stop=True,
                )
                for i in range(ue):
                    e = e0u + i
                    nc.tensor.matmul(
                        out=vw(lin, i * D, [[1, D]]),
                        lhsT=xt[:, e * BC : (e + 1) * BC],
                        rhs=wf_sb,
                        start=False,
                        stop=True,
                        skip_group_check=True,
                    )
                nc.scalar.activation(
                    out=vw(out_all, e0u * D, [[1, ue * D]]),
                    in_=lin,
                    func=Act.Relu,
                )
                # ACT's separate HWDGE ring: not FIFO-blocked behind the
                # input stream on the sync ring
                nc.scalar.dma_start(
                    out=vw(out_h[:], e0u * D, [[1, ue * D]]),
                    in_=vw(out_all, e0u * D, [[1, ue * D]]),
                )

    nc.finalize()
    return nc


def _get_nc():
    if "nc" not in _CACHE:
        _CACHE["nc"] = _build_nc()
    return _CACHE["nc"]


def _make_in_maps(self_vectors, neighbor_vectors, neighbor_relations,
                  side_embeddings, W, b):
    f16 = np.float16
    iden = np.eye(128, dtype=f16)
    ones = np.ones((1, 128), dtype=f16)
    wful = np.ascontiguousarray(np.asarray(W, dtype=f16))
    bvec = np.ascontiguousarray(np.tile(np.asarray(b, dtype=f16), 8)).reshape(1, 512)
    rel = np.asarray(neighbor_relations, dtype=f16)
    nbr = np.asarray(neighbor_vectors, dtype=f16)
    sv = np.asarray(self_vectors, dtype=f16)
    side = np.asarray(side_embeddings, dtype=np.float32)

    in_maps = []
    for cc in range(NCORES):
        sl = slice(cc * BC, (cc + 1) * BC)
        m = {}
        rc = rel[sl]                                   # [BC, E, K, D]
        ci = 0
        for ph in range(NPH):
            pe, e0 = PH[ph], PE0[ph]
            dpc = 64 // RELCH[ph]
            blk = rc[:, e0 : e0 + pe]                  # b e k d
            for c in range(RELCH[ph]):
                sub = blk[:, :, :, c * dpc : (c + 1) * dpc]  # b e k ds
                m[f"rel{ci}"] = np.ascontiguousarray(
                    sub.transpose(0, 3, 1, 2)).reshape(BC, dpc * pe * K)
                ci += 1
        nc_ = nbr[sl]                                  # [BC, E, K, D]
        ui = 0
        for ph in range(NPH):
            for eL, ue in _units(ph):
                e0 = PE0[ph] + eL
                sub = nc_[:, e0 : e0 + ue]             # b eu k d
                m[f"nbr{ui}"] = np.ascontiguousarray(
                    sub.transpose(0, 1, 3, 2)).reshape(BC, ue * D * K)
                ui += 1
        dg = np.zeros((BC, D, 32), dtype=f16)
        dg[np.arange(BC), :, np.arange(BC) % 32] = side[sl].astype(f16)
        m["dglo"] = np.ascontiguousarray(dg[:, 0:32]).reshape(BC, 32 * 32)
        m["dghi"] = np.ascontiguousarray(dg[:, 32:64]).reshape(BC, 32 * 32)
        m["selft"] = np.ascontiguousarray(sv[sl].transpose(2, 1, 0)).reshape(D, E * BC)
        m["wful"] = wful
        m["bvec"] = bvec
        m["ones"] = ones
        m["iden"] = iden
        in_maps.append(m)
    return in_maps


def kernel(self_vectors, neighbor_vectors, neighbor_relations, side_embeddings, W, b,
           _trace=False, _tmpdir=None):
    from concourse import bass_utils

    nc = _get_nc()
    in_maps = _make_in_maps(
        self_vectors, neighbor_vectors, neighbor_relations, side_embeddings, W, b
    )
    res = bass_utils.run_bass_kernel_spmd(
        nc, in_maps, list(range(NCORES)), trace=_trace, tmpdir=_tmpdir
    )
    _CACHE["last_results"] = res
    out = np.concatenate(
        [
            res.results[c]["outh"].astype(np.float32).reshape(BC, E, D)
            for c in range(NCORES)
        ],
        axis=0,
    )
    return out
